# revision 1
# baseline (speedup 1.0000x reference)
"""Trainium2 Bass kernel for nn_ODEFunc_interaction (gnn_message_passing).

Math (see reference):
  dz_dt = tanh([z, t] @ vW1 + vb1) @ vW2 + vb2                    (v-net, all rows)
  for each pair (perm[2i], perm[2i+1]):
      d_i  = z[perm[2i]] - z[perm[2i+1]]
      g_i  = grad_phi(d_i) = pW1 @ (pW2[:,0] * (1 - tanh(d_i@pW1 + pb1)^2))
      out[perm[2i]]   = dz_dt[perm[2i]]   - g_i
      out[perm[2i+1]] = dz_dt[perm[2i+1]] + g_i
  last 3 rows (triple) handled on host in float64 (tiny).

Strategy: host gathers z[perm] so each of 8 cores owns a contiguous block of
200000/8 = 25000 rows (12500 pairs). On-device layout is transposed+packed:
X[128, 6250] where partition 32*j+d holds dim d of row-chunk j (4 chunks of
6250 rows). All matmuls run as fp32r (full-rate fp32) on PE sub-tiles via
tile_position quadrants; tanh (+bias) on ACT; pair-diff and square on GPSIMD;
(1-u^2) and final +/- combine on DVE. Host scatters the result back by perm.
"""

import os
import numpy as np

B, D, H = 200003, 32, 128
NCORES = 8
P2 = 200000            # rows covered by pairs
RPC = P2 // NCORES     # 25000 rows per core
NCHUNK = 4
L = RPC // NCHUNK      # 6250 packed columns per core
LP = L + 2             # padded to keep every fp32r matmul free-size even
G = 1024               # column block (2 PSUM banks)

_CACHE = {}
LAST_RESULTS = None    # BassKernelResults of the most recent run (for test.py)


def build_program():
    """Build the single-core Bass/Tile program (same program runs SPMD on 8 cores)."""
    from contextlib import ExitStack
    import concourse.bacc as bacc
    import concourse.mybir as mybir
    import concourse.tile as tile

    dt = mybir.dt
    F32, F32R = dt.float32, dt.float32r
    AF = mybir.ActivationFunctionType
    OP = mybir.AluOpType

    F16 = dt.float16
    # All matmul streams run in fp16 (fp32r measured ~3 cyc/col on HW; fp16
    # streams at 1 cyc/col and halves the input DMA). Accuracy ~4e-4 rel.
    # One concatenated fp16 weight tensor [128, 1536]:
    #   w1rep[0:128] | pw1rep[128:256] | w2q[256:768] | pwtq[768:1280]
    #   | w1z[1280:1408] | pw1z[1408:1536]
    # w2q/pwtq are column-placed per chunk (vW2 at columns 32j of block j,
    # zeros elsewhere): matmul outputs must start at PSUM partition 0, so the
    # 4 chunk matmuls accumulate full-M into one [128,*] psum tile.
    # w1z/pw1z: chunk 3 is read from partition base 64 with K=64 and zeros in
    # rows 64:96 (partition base 96 is not encodable).
    nc = bacc.Bacc()
    X = nc.dram_tensor("x", [128, LP], F16, kind="ExternalInput")
    WC = nc.dram_tensor("wcat", [128, 2048], F16, kind="ExternalInput")
    BC = nc.dram_tensor("bias", [128, 2], F32, kind="ExternalInput")
    O = nc.dram_tensor("out", [128, LP], F32, kind="ExternalOutput")

    with tile.TileContext(nc) as tc, ExitStack() as ctx:
        wpool = ctx.enter_context(tc.tile_pool(name="wpool", bufs=1))
        xpool = ctx.enter_context(tc.tile_pool(name="xpool", bufs=4))
        upool = ctx.enter_context(tc.tile_pool(name="upool", bufs=4))
        vpool = ctx.enter_context(tc.tile_pool(name="vpool", bufs=3))
        sqpool = ctx.enter_context(tc.tile_pool(name="sqpool", bufs=3))
        dpool = ctx.enter_context(tc.tile_pool(name="dpool", bufs=2))
        qspool = ctx.enter_context(tc.tile_pool(name="qspool", bufs=2))
        opool = ctx.enter_context(tc.tile_pool(name="opool", bufs=3))
        hps = ctx.enter_context(tc.tile_pool(name="hps", bufs=2, space="PSUM"))
        dzps = ctx.enter_context(tc.tile_pool(name="dzps", bufs=1, space="PSUM"))
        aps = ctx.enter_context(tc.tile_pool(name="aps", bufs=1, space="PSUM"))
        qps = ctx.enter_context(tc.tile_pool(name="qps", bufs=1, space="PSUM"))

        wt = wpool.tile([128, 2048], F16)
        nc.sync.dma_start(wt[:], WC[:])
        bt = wpool.tile([128, 2], F32)
        nc.sync.dma_start(bt[:], BC[:])
        w1 = wt[:, 0:128]
        pw1 = wt[:, 128:256]
        w2q = wt[:, 256:768]
        pwtq = wt[:, 768:1280]      # +pW1*w2 column-placed per chunk
        pwtqn = wt[:, 1280:1792]    # negated copy (odd output columns)
        w1z = wt[:, 1792:1920]
        pw1z = wt[:, 1920:2048]
        bh = bt[:, 0:1]
        pb1 = bt[:, 1:2]

        for c0 in range(0, LP, G):
            W_ = min(G, LP - c0)
            Wp = W_ // 2
            xt = xpool.tile([128, G], F16)
            nc.sync.dma_start(xt[:, :W_], X[:, c0 : c0 + W_])

            df = dpool.tile([128, G // 2], F16)
            nc.gpsimd.tensor_tensor(df[:, :Wp], xt[:, 0:W_:2], xt[:, 1:W_:2], OP.subtract)

            dz = dzps.tile([128, G], F32)
            qp = qps.tile([128, G // 2], F32)
            ot = opool.tile([128, G], F32)

            # j=3 first: its M=64 writes (start=True) clear psum partitions
            # 64:96 to zero; j=2 then accumulates its strip on top (start=False).
            for j in (3, 0, 1, 2):
                p0 = 32 * j
                ph = hps.tile([128, G], F32)
                for s0 in range(0, W_, 512):
                    sw = min(512, W_ - s0)
                    if j == 3:
                        nc.tensor.matmul(
                            ph[:, s0 : s0 + sw],
                            w1z[64:128],
                            xt[64:128, s0 : s0 + sw],
                            start=True,
                            stop=True,
                        )
                    else:
                        nc.tensor.matmul(
                            ph[:, s0 : s0 + sw],
                            w1[p0 : p0 + 32, :],
                            xt[p0 : p0 + 32, s0 : s0 + sw],
                            start=True,
                            stop=True,
                        )
                ut = upool.tile([128, G], F16)
                nc.scalar.activation(ut[:, :W_], ph[:, :W_], AF.Tanh, bias=bh[:])
                for s0 in range(0, W_, 512):
                    sw = min(512, W_ - s0)
                    nc.tensor.matmul(
                        dz[:, s0 : s0 + sw],
                        w2q[:, H * j : H * (j + 1)],
                        ut[:, s0 : s0 + sw],
                        start=(j == 3),
                        stop=(j == 2),
                        skip_group_check=True,
                    )
                pa = aps.tile([128, G // 2], F32)
                if j == 3:
                    nc.tensor.matmul(
                        pa[:, :Wp],
                        pw1z[64:128],
                        df[64:128, :Wp],
                        start=True,
                        stop=True,
                    )
                else:
                    nc.tensor.matmul(
                        pa[:, :Wp],
                        pw1[p0 : p0 + 32, :],
                        df[p0 : p0 + 32, :Wp],
                        start=True,
                        stop=True,
                    )
                vt = vpool.tile([128, G // 2], F16)
                nc.scalar.activation(vt[:, :Wp], pa[:, :Wp], AF.Tanh, bias=pb1[:])
                sq = sqpool.tile([128, G // 2], F16)
                nc.vector.tensor_mul(sq[:, :Wp], vt[:, :Wp], vt[:, :Wp])
                # q = pwtq^T v^2 accumulated over the 4 chunks; the constant
                # part of g = pwtq^T (1 - v^2) is folded on the host.
                nc.tensor.matmul(
                    qp[:, :Wp],
                    pwtq[:, H * j : H * (j + 1)],
                    sq[:, :Wp],
                    start=(j == 3),
                    stop=(j == 2),
                    skip_group_check=True,
                )

            qs = qspool.tile([128, G // 2], F32)
            nc.vector.tensor_copy(qs[:, :Wp], qp[:, :Wp])
            nc.vector.tensor_tensor(ot[:, 0:W_:2], dz[:, 0:W_:2], qs[:, :Wp], OP.add)
            nc.vector.tensor_tensor(ot[:, 1:W_:2], dz[:, 1:W_:2], qs[:, :Wp], OP.subtract)
            nc.sync.dma_start(O[:, c0 : c0 + W_], ot[:, :W_])

    nc.compile()
    return nc


def _prep_weights(t, vW1, vb1, vW2, vb2, pW1, pb1, pW2):
    f32 = np.float32
    t = np.asarray(t, dtype=f32).reshape(-1)[0]
    vW1 = np.asarray(vW1, dtype=f32)
    w1rep = np.tile(np.ascontiguousarray(vW1[:32]), (4, 1))            # [128,128]
    biash = (np.asarray(vb1, f32) + t * vW1[32]).reshape(128, 1).astype(f32)
    vw2 = np.ascontiguousarray(np.asarray(vW2, f32))                   # [128,32]
    pW1 = np.asarray(pW1, f32)
    pw1rep = np.tile(pW1, (4, 1))                                      # [128,128]
    pb1c = np.asarray(pb1, f32).reshape(128, 1).copy()
    w2col = np.asarray(pW2, f32).reshape(128)
    pw1tw2 = np.ascontiguousarray((pW1 * w2col[None, :]).T)            # [128,32]
    z96 = np.zeros((96, 128), f32)
    w2q = np.zeros((128, 512), f32)
    pwtq = np.zeros((128, 512), f32)
    for j in range(4):
        w2q[:, 128 * j + 32 * j : 128 * j + 32 * j + 32] = vw2
        pwtq[:, 128 * j + 32 * j : 128 * j + 32 * j + 32] = pw1tw2
    w1z = np.vstack([z96, vW1[:32]])                                   # [128,128]
    pw1z = np.vstack([z96, pW1])                                       # [128,128]
    wcat = np.hstack([w1rep, pw1rep, w2q, pwtq, -pwtq, w1z, pw1z]).astype(np.float16)
    bias = np.hstack([biash, pb1c]).astype(f32)
    # constant part of g: c0[d] = sum_k pW1[d,k]*w2[k], in the fp16 weight
    # precision actually used on device
    c0base = pw1tw2.astype(np.float16).astype(f32).sum(axis=0)         # [32]
    return {"wcat": np.ascontiguousarray(wcat), "bias": np.ascontiguousarray(bias),
            "_c0base": c0base}


def _pack_core(zc):
    """[25000, 32] f32 -> [128, 6252] fp16 packed (partition 32*j+d, col i =
    row j*L+i), padded with 2 zero columns."""
    out = np.zeros((128, LP), dtype=np.float16)
    out[:, :L] = zc.reshape(NCHUNK, L, 32).transpose(0, 2, 1).reshape(128, L)
    return out


def _unpack_core(oc):
    """[128, 6252] packed -> [25000, 32]."""
    return oc[:, :L].reshape(NCHUNK, 32, L).transpose(0, 2, 1).reshape(RPC, 32)


def _host_triple(t, z3, vW1, vb1, vW2, vb2, pW1, pb1, pW2):
    """Exact float64 computation of the 3 leftover rows: dz_dt + triple forces."""
    f8 = np.float64
    z3 = z3.astype(f8)
    vW1 = np.asarray(vW1, f8)
    t = float(np.asarray(t).reshape(-1)[0])
    h3 = np.tanh(z3 @ vW1[:32] + t * vW1[32] + np.asarray(vb1, f8))
    dz3 = h3 @ np.asarray(vW2, f8) + np.asarray(vb2, f8)

    pW1 = np.asarray(pW1, f8)
    w2 = np.asarray(pW2, f8).reshape(128)
    d9 = (z3[:, None, :] - z3[None, :, :]).reshape(9, 32)
    u9 = np.tanh(d9 @ pW1 + np.asarray(pb1, f8))
    s9 = (1.0 - u9 * u9) * w2[None, :]
    g9 = s9 @ pW1.T                       # grad_phi rows
    f9 = (-g9).reshape(3, 3, 32)
    f9 = f9 * (1.0 - np.eye(3)[:, :, None])
    force3 = f9.sum(axis=1) * 2.0
    return (dz3 + force3).astype(np.float32)


def kernel(t, z, perm, vW1, vb1, vW2, vb2, pW1, pb1, pW2, pb2):
    from concourse.bass_utils import run_bass_kernel_spmd

    global LAST_RESULTS
    if "nc" not in _CACHE:
        _CACHE["nc"] = build_program()
    nc = _CACHE["nc"]

    z = np.asarray(z, np.float32)
    perm = np.asarray(perm)
    weights = _prep_weights(t, vW1, vb1, vW2, vb2, pW1, pb1, pW2)

    c0base = weights.pop("_c0base")
    zg = z[perm[:P2]]                       # [200000, 32] gathered pair rows
    in_maps = []
    for c in range(NCORES):
        im = {"x": _pack_core(zg[c * RPC : (c + 1) * RPC])}
        im.update(weights)
        in_maps.append(im)

    trace = bool(int(os.environ.get("KERNEL_TRACE", "0")))
    res = run_bass_kernel_spmd(nc, in_maps, list(range(NCORES)), trace=trace)
    LAST_RESULTS = res

    out = np.empty((B, 32), dtype=np.float32)
    og = np.concatenate([_unpack_core(res.results[c]["out"]) for c in range(NCORES)], axis=0)
    vb2f = np.asarray(vb2, np.float32)
    og[0::2] += (vb2f - c0base)[None, :]
    og[1::2] += (vb2f + c0base)[None, :]
    out[perm[:P2]] = og
    out[perm[P2:]] = _host_triple(t, z[perm[P2:]], vW1, vb1, vW2, vb2, pW1, pb1, pW2)
    return out



# revision 9
# speedup vs baseline: 1.0313x; 1.0313x over previous
"""Trainium2 Bass kernel for nn_ODEFunc_interaction (gnn_message_passing).

Math (see reference):
  dz_dt = tanh([z, t] @ vW1 + vb1) @ vW2 + vb2                    (v-net, all rows)
  for each pair (perm[2i], perm[2i+1]):
      d_i  = z[perm[2i]] - z[perm[2i+1]]
      v    = tanh(d_i @ pW1 + pb1);  q = (pW1*pW2)^T v^2;  c0 = sum(pW1*pW2)
      out[perm[2i]]   = dz_dt[perm[2i]]   + q - c0   (+vb2)
      out[perm[2i+1]] = dz_dt[perm[2i+1]] - q + c0   (+vb2)
  last 3 rows (triple) handled on host in float64 (tiny).

Mapping: 8 cores, data-parallel over pairs. Per core 25000 rows = 12500 pairs,
packed 4 chunks deep in the partition dim: partition 32j+d = dim d of chunk j.
Column space (per chunk) = 3136 padded pairs in superblocks of 512 pairs
(tail 64), each superblock ordered [wave0-even 256 | wave0-odd 256 | wave1-e |
wave1-o].

PE array is addressed as 32-wide tiles (tile_position): the 4 chunk h/pa
matmuls (K=32) run as concurrent row-tiles (32j,0); the 4 dz/q matmuls
(M=32) run as concurrent col-tiles (0,32j) with shared stationary weights.
Each in-flight matmul owns a full PSUM bank (concurrent drains into one
bank wedge the device). q accumulates into the dz PSUM region with +/-
weights, so the final combine is one DVE psum->sbuf fp16 copy. The dz
tile reuses the pa PSUM banks (pool rotation) to fit 8 banks exactly.
"""

import os
import numpy as np

B, D, H = 200003, 32, 128
NCORES = 8
P2 = 200000              # rows covered by pairs
RPC = P2 // NCORES       # 25000 rows per core
NCHUNK = 4
ROWS_PC = RPC // NCHUNK  # 6250 rows per chunk
PAIRS_PC = ROWS_PC // 2  # 3125 pairs per chunk
HALFP = 3136             # padded pairs per chunk: 6*512 + 64
SBW = 512                # pairs per superblock
SBS = [SBW] * 6 + [HALFP - 6 * SBW]
XC = 2 * HALFP           # 6272 columns per core

_CACHE = {}
LAST_RESULTS = None      # BassKernelResults of the most recent run (for test.py)


def build_program():
    from contextlib import ExitStack
    import concourse.bacc as bacc
    import concourse.mybir as mybir
    import concourse.tile as tile

    dt = mybir.dt
    F16, F32 = dt.float16, dt.float32
    AF = mybir.ActivationFunctionType
    OP = mybir.AluOpType

    nc = bacc.Bacc()
    X = nc.dram_tensor("x", [128, XC], F16, kind="ExternalInput")
    WT = nc.dram_tensor("wcat", [128, 352], F16, kind="ExternalInput")
    BT = nc.dram_tensor("bias", [128, 2], F32, kind="ExternalInput")
    O = nc.dram_tensor("out", [128, XC], F16, kind="ExternalOutput")

    with tile.TileContext(nc) as tc, ExitStack() as ctx:
        wpool = ctx.enter_context(tc.tile_pool(name="wpool", bufs=1))
        xpool = ctx.enter_context(tc.tile_pool(name="xpool", bufs=3))
        dfpool = ctx.enter_context(tc.tile_pool(name="dfpool", bufs=2))
        upool = ctx.enter_context(tc.tile_pool(name="upool", bufs=3))
        vpool = ctx.enter_context(tc.tile_pool(name="vpool", bufs=2))
        sqpool = ctx.enter_context(tc.tile_pool(name="sqpool", bufs=2))
        opool = ctx.enter_context(tc.tile_pool(name="opool", bufs=3))
        # PSUM: hps 4 banks; papool 4 banks shared by pa pre-acts and dz/q
        hps = ctx.enter_context(tc.tile_pool(name="hps", bufs=1, space="PSUM"))
        papool = ctx.enter_context(tc.tile_pool(name="papool", bufs=1, space="PSUM"))

        wt = wpool.tile([128, 352], F16)
        nc.sync.dma_start(wt[:], WT[:])
        bt = wpool.tile([128, 2], F32)
        nc.sync.dma_start(bt[:], BT[:])
        w1 = wt[:, 0:128]      # [32j+d, h] = vW1[d, h]
        pw1 = wt[:, 128:256]   # [32j+d, h] = pW1[d, h]
        w2 = wt[:, 256:288]    # [h, d] = vW2[h, d]
        pwp = wt[:, 288:320]   # [h, d] = pW1[d, h] * pW2[h]
        pwn = wt[:, 320:352]   # -pwp
        bh = bt[:, 0:1]        # vb1 + t * vW1[32]
        pb1 = bt[:, 1:2]

        c0 = 0
        for W in SBS:
            W2_ = 2 * W
            waves = [(w, 256) for w in range(W // 256)] if W >= 256 else [(0, W)]
            xt = xpool.tile([128, 2 * SBW], F16)
            nc.sync.dma_start(xt[:, :W2_], X[:, c0 : c0 + W2_])

            # diffs, wave-major pair columns: df[:, 256w+i] = e - o
            dft = dfpool.tile([128, SBW], F16)
            for w, V in waves:
                nc.gpsimd.tensor_tensor(
                    dft[:, 256 * w : 256 * w + V],
                    xt[:, 512 * w : 512 * w + V],
                    xt[:, 512 * w + V : 512 * w + 2 * V],
                    OP.subtract,
                )

            uts = []
            for w, V in waves:
                # h pre-acts for wave w: 4 concurrent row-tiles, chunk j ->
                # psum bank j (cols 512j), each matmul sole writer of its bank
                ph = hps.tile([128, 2048], F32)
                for j in range(NCHUNK):
                    p0 = 32 * j
                    nc.tensor.matmul(
                        ph[:, 512 * j : 512 * j + 2 * V],
                        w1[p0 : p0 + 32, :],
                        xt[p0 : p0 + 32, 512 * w : 512 * w + 2 * V],
                        start=True, stop=True,
                        tile_position=(p0, 0),
                    )
                ut = upool.tile([128, 2048], F16)
                if V == 256:
                    nc.scalar.activation(ut[:, :], ph[:, :], AF.Tanh, bias=bh[:])
                else:
                    for j in range(NCHUNK):
                        nc.scalar.activation(
                            ut[:, 512 * j : 512 * j + 2 * V],
                            ph[:, 512 * j : 512 * j + 2 * V],
                            AF.Tanh, bias=bh[:],
                        )
                uts.append(ut)

            # pair pre-acts: 4 concurrent row-tiles, chunk j -> bank j
            pap = papool.tile([128, 2048], F32, tag="pz")
            for j in range(NCHUNK):
                p0 = 32 * j
                nc.tensor.matmul(
                    pap[:, 512 * j : 512 * j + W],
                    pw1[p0 : p0 + 32, :],
                    dft[p0 : p0 + 32, :W],
                    start=True, stop=True,
                    tile_position=(p0, 0),
                )
            vt = vpool.tile([128, 2048], F16)
            sq = sqpool.tile([128, 2048], F16)
            if W == SBW:
                nc.scalar.activation(vt[:, :], pap[:, :], AF.Tanh, bias=pb1[:])
                nc.gpsimd.tensor_tensor(sq[:, :], vt[:, :], vt[:, :], OP.mult)
            else:
                for j in range(NCHUNK):
                    nc.scalar.activation(
                        vt[:, 512 * j : 512 * j + W],
                        pap[:, 512 * j : 512 * j + W],
                        AF.Tanh, bias=pb1[:],
                    )
                for j in range(NCHUNK):
                    nc.gpsimd.tensor_tensor(
                        sq[:, 512 * j : 512 * j + W],
                        vt[:, 512 * j : 512 * j + W],
                        vt[:, 512 * j : 512 * j + W],
                        OP.mult,
                    )

            # dz + q accumulate in a fresh tile on the pa banks (pool reuse).
            # col layout = x layout: wave w even at 512w, odd at 512w+256.
            dzp = papool.tile([128, 2048], F32, tag="pz")
            for (w, V), ut in zip(waves, uts):
                for j in range(NCHUNK):
                    p0 = 32 * j
                    nc.tensor.matmul(
                        dzp[p0 : p0 + 32, 512 * w : 512 * w + 2 * V],
                        w2[:, :],
                        ut[:, 512 * j : 512 * j + 2 * V],
                        start=True, stop=False,
                        tile_position=(0, p0),
                        skip_group_check=True,
                    )
            for (w, V), _ in zip(waves, uts):
                for j in range(NCHUNK):
                    p0 = 32 * j
                    nc.tensor.matmul(
                        dzp[p0 : p0 + 32, 512 * w : 512 * w + V],
                        pwp[:, :],
                        sq[:, 512 * j + 256 * w : 512 * j + 256 * w + V],
                        start=False, stop=False,
                        tile_position=(0, p0),
                        skip_group_check=True,
                    )
            nwaves = len(waves)
            for wi, ((w, V), _) in enumerate(zip(waves, uts)):
                for j in range(NCHUNK):
                    p0 = 32 * j
                    nc.tensor.matmul(
                        dzp[p0 : p0 + 32, 512 * w + V : 512 * w + 2 * V],
                        pwn[:, :],
                        sq[:, 512 * j + 256 * w : 512 * j + 256 * w + V],
                        start=False, stop=(wi == nwaves - 1),
                        tile_position=(0, p0),
                        skip_group_check=True,
                    )

            ot = opool.tile([128, 2 * SBW], F16)
            nc.vector.tensor_copy(ot[:, :W2_], dzp[:, :W2_])
            nc.sync.dma_start(O[:, c0 : c0 + W2_], ot[:, :W2_])
            c0 += W2_

    nc.compile()
    return nc


def _prep_weights(t, vW1, vb1, vW2, vb2, pW1, pb1, pW2):
    f32 = np.float32
    t = np.asarray(t, dtype=f32).reshape(-1)[0]
    vW1 = np.asarray(vW1, f32)
    w1rep = np.tile(np.ascontiguousarray(vW1[:32]), (4, 1))            # [128,128]
    biash = (np.asarray(vb1, f32) + t * vW1[32]).reshape(128, 1).astype(f32)
    pW1 = np.asarray(pW1, f32)
    pw1rep = np.tile(pW1, (4, 1))                                      # [128,128]
    pb1c = np.asarray(pb1, f32).reshape(128, 1).copy()
    w2 = np.ascontiguousarray(np.asarray(vW2, f32))                    # [128,32]
    w2col = np.asarray(pW2, f32).reshape(128)
    pwp = np.ascontiguousarray(pW1.T * w2col[:, None])                 # [128,32]
    wcat = np.hstack([w1rep, pw1rep, w2, pwp, -pwp]).astype(np.float16)
    bias = np.hstack([biash, pb1c]).astype(f32)
    # constant part of q: c0[d] = sum_h pW1[d,h]*pW2[h], in fp16 weight precision
    c0base = wcat[:, 288:320].astype(f32).sum(axis=0)                  # [32]
    return {"wcat": np.ascontiguousarray(wcat), "bias": np.ascontiguousarray(bias),
            "_c0base": c0base}


def _pack_core(zc):
    """[25000, 32] f32 -> [128, 6272] fp16: partition 32j+d = dim d of chunk j;
    cols per superblock: [w0-even 256 | w0-odd 256 | w1-e 256 | w1-o 256]."""
    zp = np.zeros((NCHUNK, HALFP, 2, 32), dtype=np.float16)
    zp[:, :PAIRS_PC] = zc.reshape(NCHUNK, PAIRS_PC, 2, 32)
    # full superblocks: [4, 6, 2(wave), 256, 2(half), 32]
    full = zp[:, : 6 * SBW].reshape(NCHUNK, 6, 2, 256, 2, 32)
    out = np.empty((128, XC), dtype=np.float16)
    out[:, : 12 * SBW] = full.transpose(0, 5, 1, 2, 4, 3).reshape(128, 12 * SBW)
    tail = zp[:, 6 * SBW :]                                 # [4, 64, 2, 32]
    out[:, 12 * SBW :] = tail.transpose(0, 3, 2, 1).reshape(128, 2 * (HALFP - 6 * SBW))
    return out


def _unpack_core(oc):
    """[128, 6272] fp16 -> even [4,HALFP,32], odd [4,HALFP,32] (f32)."""
    T = HALFP - 6 * SBW
    full = (
        oc[:, : 12 * SBW].astype(np.float32)
        .reshape(NCHUNK, 32, 6, 2, 2, 256)
        .transpose(0, 2, 3, 5, 4, 1)                        # [4, 6, wave, 256, half, 32]
        .reshape(NCHUNK, 6 * SBW, 2, 32)
    )
    tail = (
        oc[:, 12 * SBW :].astype(np.float32)
        .reshape(NCHUNK, 32, 2, T)
        .transpose(0, 3, 2, 1)                              # [4, T, 2, 32]
    )
    ev = np.concatenate([full[:, :, 0], tail[:, :, 0]], axis=1)
    od = np.concatenate([full[:, :, 1], tail[:, :, 1]], axis=1)
    return ev, od


def _host_triple(t, z3, vW1, vb1, vW2, vb2, pW1, pb1, pW2):
    """Exact float64 computation of the 3 leftover rows: dz_dt + triple forces."""
    f8 = np.float64
    z3 = z3.astype(f8)
    vW1 = np.asarray(vW1, f8)
    t = float(np.asarray(t).reshape(-1)[0])
    h3 = np.tanh(z3 @ vW1[:32] + t * vW1[32] + np.asarray(vb1, f8))
    dz3 = h3 @ np.asarray(vW2, f8) + np.asarray(vb2, f8)

    pW1 = np.asarray(pW1, f8)
    w2 = np.asarray(pW2, f8).reshape(128)
    d9 = (z3[:, None, :] - z3[None, :, :]).reshape(9, 32)
    u9 = np.tanh(d9 @ pW1 + np.asarray(pb1, f8))
    s9 = (1.0 - u9 * u9) * w2[None, :]
    g9 = s9 @ pW1.T
    f9 = (-g9).reshape(3, 3, 32)
    f9 = f9 * (1.0 - np.eye(3)[:, :, None])
    force3 = f9.sum(axis=1) * 2.0
    return (dz3 + force3).astype(np.float32)


def kernel(t, z, perm, vW1, vb1, vW2, vb2, pW1, pb1, pW2, pb2):
    from concourse.bass_utils import run_bass_kernel_spmd

    global LAST_RESULTS
    if "nc" not in _CACHE:
        _CACHE["nc"] = build_program()
    nc = _CACHE["nc"]

    z = np.asarray(z, np.float32)
    perm = np.asarray(perm)
    weights = _prep_weights(t, vW1, vb1, vW2, vb2, pW1, pb1, pW2)
    c0base = weights.pop("_c0base")

    zg = z[perm[:P2]]                       # [200000, 32] gathered pair rows
    in_maps = []
    for c in range(NCORES):
        im = {"x": _pack_core(zg[c * RPC : (c + 1) * RPC])}
        im.update(weights)
        in_maps.append(im)

    trace = bool(int(os.environ.get("KERNEL_TRACE", "0")))
    res = run_bass_kernel_spmd(nc, in_maps, list(range(NCORES)), trace=trace)
    LAST_RESULTS = res

    vb2f = np.asarray(vb2, np.float32)
    add_e = (vb2f - c0base)[None, :]
    add_o = (vb2f + c0base)[None, :]
    out = np.empty((B, 32), dtype=np.float32)
    og = np.empty((RPC * NCORES, 32), dtype=np.float32)
    for c in range(NCORES):
        ev, od = _unpack_core(res.results[c]["out"])
        blk = np.empty((NCHUNK, PAIRS_PC, 2, 32), dtype=np.float32)
        blk[:, :, 0] = ev[:, :PAIRS_PC] + add_e
        blk[:, :, 1] = od[:, :PAIRS_PC] + add_o
        og[c * RPC : (c + 1) * RPC] = blk.reshape(RPC, 32)
    out[perm[:P2]] = og
    out[perm[P2:]] = _host_triple(t, z[perm[P2:]], vW1, vb1, vW2, vb2, pW1, pb1, pW2)
    return out


# revision 12
# speedup vs baseline: 1.3327x; 1.2922x over previous
"""Trainium2 Bass kernel for nn_ODEFunc_interaction (gnn_message_passing).

Math (see reference):
  dz_dt = tanh([z, t] @ vW1 + vb1) @ vW2 + vb2                    (v-net, all rows)
  for each pair (perm[2i], perm[2i+1]):
      d_i  = z[perm[2i]] - z[perm[2i+1]]
      v    = tanh(d_i @ pW1 + pb1);  q = (pW1*pW2)^T v^2;  c0 = sum(pW1*pW2)
      out[perm[2i]]   = dz_dt[perm[2i]]   + q - c0   (+vb2)
      out[perm[2i+1]] = dz_dt[perm[2i+1]] - q + c0   (+vb2)
  last 3 rows (triple) handled on host in float64 (tiny).

Mapping: 8 cores, data-parallel over pairs. Per core 25000 rows = 12500 pairs,
packed 4 chunks deep in the partition dim: partition 32j+d = dim d of chunk j.
Column space (per chunk) = 3136 padded pairs in superblocks of 512 pairs
(tail 64), each superblock ordered [wave0-even 256 | wave0-odd 256 | wave1-e |
wave1-o].

PE array is addressed as 32-wide tiles (tile_position): the 4 chunk h/pa
matmuls (K=32) run as concurrent row-tiles (32j,0); the 4 dz/q matmuls
(M=32) run as concurrent col-tiles (0,32j) with shared stationary weights.
Each in-flight matmul owns a full PSUM bank (concurrent drains into one
bank wedge the device). q accumulates into the dz PSUM region with +/-
weights, so the final combine is one DVE psum->sbuf fp16 copy. The dz
tile reuses the pa PSUM banks (pool rotation) to fit 8 banks exactly.
"""

import os
import numpy as np

B, D, H = 200003, 32, 128
NCORES = 8
P2 = 200000              # rows covered by pairs
RPC = P2 // NCORES       # 25000 rows per core
NCHUNK = 4
ROWS_PC = RPC // NCHUNK  # 6250 rows per chunk
PAIRS_PC = ROWS_PC // 2  # 3125 pairs per chunk
HALFP = 3136             # padded pairs per chunk: 6*512 + 64
SBW = 512                # pairs per superblock
SBS = [SBW] * 6 + [HALFP - 6 * SBW]
XC = 2 * HALFP           # 6272 columns per core

_CACHE = {}
LAST_RESULTS = None      # BassKernelResults of the most recent run (for test.py)


def build_program():
    from contextlib import ExitStack
    import concourse.bacc as bacc
    import concourse.mybir as mybir
    import concourse.tile as tile

    dt = mybir.dt
    F16, F32 = dt.float16, dt.float32
    AF = mybir.ActivationFunctionType
    OP = mybir.AluOpType

    nc = bacc.Bacc()
    X = nc.dram_tensor("x", [128, XC], F16, kind="ExternalInput")
    WT = nc.dram_tensor("wcat", [128, 352], F16, kind="ExternalInput")
    BT = nc.dram_tensor("bias", [128, 2], F32, kind="ExternalInput")
    O = nc.dram_tensor("out", [128, XC], F16, kind="ExternalOutput")

    with tile.TileContext(nc) as tc, ExitStack() as ctx:
        wpool = ctx.enter_context(tc.tile_pool(name="wpool", bufs=1))
        xpool = ctx.enter_context(tc.tile_pool(name="xpool", bufs=3))
        dfpool = ctx.enter_context(tc.tile_pool(name="dfpool", bufs=2))
        upool = ctx.enter_context(tc.tile_pool(name="upool", bufs=3))
        vpool = ctx.enter_context(tc.tile_pool(name="vpool", bufs=2))
        sqpool = ctx.enter_context(tc.tile_pool(name="sqpool", bufs=2))
        opool = ctx.enter_context(tc.tile_pool(name="opool", bufs=3))
        # PSUM: ph 4 banks + pa 2 banks + dz 2 banks = 8 exactly
        hps = ctx.enter_context(tc.tile_pool(name="hps", bufs=1, space="PSUM"))
        papool = ctx.enter_context(tc.tile_pool(name="papool", bufs=1, space="PSUM"))
        dzpool = ctx.enter_context(tc.tile_pool(name="dzpool", bufs=1, space="PSUM"))

        wt = wpool.tile([128, 352], F16)
        nc.sync.dma_start(wt[:], WT[:])
        bt = wpool.tile([128, 2], F32)
        nc.sync.dma_start(bt[:], BT[:])
        w1 = wt[:, 0:128]      # [32j+d, h] = vW1[d, h]
        pw1 = wt[:, 128:256]   # [32j+d, h] = pW1[d, h]
        w2 = wt[:, 256:288]    # [h, d] = vW2[h, d]
        pwp = wt[:, 288:320]   # [h, d] = pW1[d, h] * pW2[h]
        pwn = wt[:, 320:352]   # -pwp
        bh = bt[:, 0:1]        # vb1 + t * vW1[32]
        pb1 = bt[:, 1:2]

        def pa_mms(half, dft, W):
            """pa pre-acts for chunks (2*half, 2*half+1) -> [128,1024] psum,
            chunk c at cols 512c (sole-writer bank per matmul)."""
            pap = papool.tile([128, 1024], F32, tag="pap", name=f"pap{half}")
            for c in range(2):
                j = 2 * half + c
                p0 = 32 * j
                nc.tensor.matmul(
                    pap[:, 512 * c : 512 * c + W],
                    pw1[p0 : p0 + 32, :],
                    dft[p0 : p0 + 32, :W],
                    start=True, stop=True,
                    tile_position=(p0, 0),
                )
            return pap

        def pa_act(half, pap, W):
            """tanh then square (DVE fp16 2x) for one pa half."""
            vt = vpool.tile([128, 1024], F16, tag="vt", name=f"vt{half}")
            sq = sqpool.tile([128, 1024], F16, tag="sq", name=f"sq{half}")
            if W == SBW:
                nc.scalar.activation(vt[:, :], pap[:, :], AF.Tanh, bias=pb1[:])
                nc.vector.tensor_tensor(sq[:, :], vt[:, :], vt[:, :], OP.mult)
            else:
                for c in range(2):
                    nc.scalar.activation(
                        vt[:, 512 * c : 512 * c + W],
                        pap[:, 512 * c : 512 * c + W],
                        AF.Tanh, bias=pb1[:],
                    )
                for c in range(2):
                    nc.vector.tensor_tensor(
                        sq[:, 512 * c : 512 * c + W],
                        vt[:, 512 * c : 512 * c + W],
                        vt[:, 512 * c : 512 * c + W],
                        OP.mult,
                    )
            return sq

        def h_mms(xt, w, V, k):
            """h pre-acts for wave w: 4 concurrent K=32 row-tiles, chunk j ->
            psum bank j (cols 512j), each matmul sole writer of its bank."""
            ph = hps.tile([128, 2048], F32, tag="ph", name=f"ph{k}_{w}")
            for j in range(NCHUNK):
                p0 = 32 * j
                nc.tensor.matmul(
                    ph[:, 512 * j : 512 * j + 2 * V],
                    w1[p0 : p0 + 32, :],
                    xt[p0 : p0 + 32, 512 * w : 512 * w + 2 * V],
                    start=True, stop=True,
                    tile_position=(p0, 0),
                )
            return ph

        def h_act(ph, w, V, k):
            ut = upool.tile([128, 2048], F16, tag="ut", name=f"ut{k}_{w}")
            if V == 256:
                nc.scalar.activation(ut[:, :], ph[:, :], AF.Tanh, bias=bh[:])
            else:
                for j in range(NCHUNK):
                    nc.scalar.activation(
                        ut[:, 512 * j : 512 * j + 2 * V],
                        ph[:, 512 * j : 512 * j + 2 * V],
                        AF.Tanh, bias=bh[:],
                    )
            return ut

        c0 = 0
        for k, W in enumerate(SBS):
            W2_ = 2 * W
            waves = [(w, 256) for w in range(W // 256)] if W >= 256 else [(0, W)]
            xt = xpool.tile([128, 2 * SBW], F16)
            nc.sync.dma_start(xt[:, :W2_], X[:, c0 : c0 + W2_])

            # diffs, wave-major pair columns: df[:, 256w+i] = e - o
            dft = dfpool.tile([128, SBW], F16)
            for w, V in waves:
                nc.gpsimd.tensor_tensor(
                    dft[:, 256 * w : 256 * w + V],
                    xt[:, 512 * w : 512 * w + V],
                    xt[:, 512 * w + V : 512 * w + 2 * V],
                    OP.subtract,
                )

            # engine queue orders (independent per engine):
            #   PE : h(w0), paA, paB, h(w1), dz, q   (3 heavy LDW configs)
            #   ACT: th(w0), tvA, th(w1), tvB        (tvB fills the SB gap)
            uts = []
            ph0 = h_mms(xt, waves[0][0], waves[0][1], k)
            uts.append(h_act(ph0, waves[0][0], waves[0][1], k))
            papA = pa_mms(0, dft, W)
            sqA = pa_act(0, papA, W)
            papB = pa_mms(1, dft, W)
            if len(waves) > 1:
                w, V = waves[1]
                ph1 = h_mms(xt, w, V, k)
                uts.append(h_act(ph1, w, V, k))
            sqB = pa_act(1, papB, W)

            # dz + q accumulate in [128,1024]: wave w -> bank w (cols 512w)
            dzp = dzpool.tile([128, 1024], F32, tag="dzp")
            for (w, V), ut in zip(waves, uts):
                for j in range(NCHUNK):
                    p0 = 32 * j
                    nc.tensor.matmul(
                        dzp[p0 : p0 + 32, 512 * w : 512 * w + 2 * V],
                        w2[:, :],
                        ut[:, 512 * j : 512 * j + 2 * V],
                        start=True, stop=False,
                        tile_position=(0, p0),
                        skip_group_check=True,
                    )
            nwaves = len(waves)
            for sgn, pw in ((0, pwp), (1, pwn)):
                for wi, (w, V) in enumerate(waves):
                    for j in range(NCHUNK):
                        p0 = 32 * j
                        sqx = sqA if j < 2 else sqB
                        nc.tensor.matmul(
                            dzp[p0 : p0 + 32, 512 * w + sgn * V : 512 * w + (sgn + 1) * V],
                            pw[:, :],
                            sqx[:, 512 * (j % 2) + 256 * w : 512 * (j % 2) + 256 * w + V],
                            start=False, stop=(sgn == 1 and wi == nwaves - 1),
                            tile_position=(0, p0),
                            skip_group_check=True,
                        )

            ot = opool.tile([128, 2 * SBW], F16)
            nc.vector.tensor_copy(ot[:, :W2_], dzp[:, :W2_])
            nc.sync.dma_start(O[:, c0 : c0 + W2_], ot[:, :W2_])
            c0 += W2_

    nc.compile()
    return nc


def _prep_weights(t, vW1, vb1, vW2, vb2, pW1, pb1, pW2):
    f32 = np.float32
    t = np.asarray(t, dtype=f32).reshape(-1)[0]
    vW1 = np.asarray(vW1, f32)
    w1rep = np.tile(np.ascontiguousarray(vW1[:32]), (4, 1))            # [128,128]
    biash = (np.asarray(vb1, f32) + t * vW1[32]).reshape(128, 1).astype(f32)
    pW1 = np.asarray(pW1, f32)
    pw1rep = np.tile(pW1, (4, 1))                                      # [128,128]
    pb1c = np.asarray(pb1, f32).reshape(128, 1).copy()
    w2 = np.ascontiguousarray(np.asarray(vW2, f32))                    # [128,32]
    w2col = np.asarray(pW2, f32).reshape(128)
    pwp = np.ascontiguousarray(pW1.T * w2col[:, None])                 # [128,32]
    wcat = np.hstack([w1rep, pw1rep, w2, pwp, -pwp]).astype(np.float16)
    bias = np.hstack([biash, pb1c]).astype(f32)
    # constant part of q: c0[d] = sum_h pW1[d,h]*pW2[h], in fp16 weight precision
    c0base = wcat[:, 288:320].astype(f32).sum(axis=0)                  # [32]
    return {"wcat": np.ascontiguousarray(wcat), "bias": np.ascontiguousarray(bias),
            "_c0base": c0base}


def _pack_core(zc):
    """[25000, 32] f32 -> [128, 6272] fp16: partition 32j+d = dim d of chunk j;
    cols per superblock: [w0-even 256 | w0-odd 256 | w1-e 256 | w1-o 256]."""
    zp = np.zeros((NCHUNK, HALFP, 2, 32), dtype=np.float16)
    zp[:, :PAIRS_PC] = zc.reshape(NCHUNK, PAIRS_PC, 2, 32)
    # full superblocks: [4, 6, 2(wave), 256, 2(half), 32]
    full = zp[:, : 6 * SBW].reshape(NCHUNK, 6, 2, 256, 2, 32)
    out = np.empty((128, XC), dtype=np.float16)
    out[:, : 12 * SBW] = full.transpose(0, 5, 1, 2, 4, 3).reshape(128, 12 * SBW)
    tail = zp[:, 6 * SBW :]                                 # [4, 64, 2, 32]
    out[:, 12 * SBW :] = tail.transpose(0, 3, 2, 1).reshape(128, 2 * (HALFP - 6 * SBW))
    return out


def _unpack_core(oc):
    """[128, 6272] fp16 -> even [4,HALFP,32], odd [4,HALFP,32] (f32)."""
    T = HALFP - 6 * SBW
    full = (
        oc[:, : 12 * SBW].astype(np.float32)
        .reshape(NCHUNK, 32, 6, 2, 2, 256)
        .transpose(0, 2, 3, 5, 4, 1)                        # [4, 6, wave, 256, half, 32]
        .reshape(NCHUNK, 6 * SBW, 2, 32)
    )
    tail = (
        oc[:, 12 * SBW :].astype(np.float32)
        .reshape(NCHUNK, 32, 2, T)
        .transpose(0, 3, 2, 1)                              # [4, T, 2, 32]
    )
    ev = np.concatenate([full[:, :, 0], tail[:, :, 0]], axis=1)
    od = np.concatenate([full[:, :, 1], tail[:, :, 1]], axis=1)
    return ev, od


def _host_triple(t, z3, vW1, vb1, vW2, vb2, pW1, pb1, pW2):
    """Exact float64 computation of the 3 leftover rows: dz_dt + triple forces."""
    f8 = np.float64
    z3 = z3.astype(f8)
    vW1 = np.asarray(vW1, f8)
    t = float(np.asarray(t).reshape(-1)[0])
    h3 = np.tanh(z3 @ vW1[:32] + t * vW1[32] + np.asarray(vb1, f8))
    dz3 = h3 @ np.asarray(vW2, f8) + np.asarray(vb2, f8)

    pW1 = np.asarray(pW1, f8)
    w2 = np.asarray(pW2, f8).reshape(128)
    d9 = (z3[:, None, :] - z3[None, :, :]).reshape(9, 32)
    u9 = np.tanh(d9 @ pW1 + np.asarray(pb1, f8))
    s9 = (1.0 - u9 * u9) * w2[None, :]
    g9 = s9 @ pW1.T
    f9 = (-g9).reshape(3, 3, 32)
    f9 = f9 * (1.0 - np.eye(3)[:, :, None])
    force3 = f9.sum(axis=1) * 2.0
    return (dz3 + force3).astype(np.float32)


def kernel(t, z, perm, vW1, vb1, vW2, vb2, pW1, pb1, pW2, pb2):
    from concourse.bass_utils import run_bass_kernel_spmd

    global LAST_RESULTS
    if "nc" not in _CACHE:
        _CACHE["nc"] = build_program()
    nc = _CACHE["nc"]

    z = np.asarray(z, np.float32)
    perm = np.asarray(perm)
    weights = _prep_weights(t, vW1, vb1, vW2, vb2, pW1, pb1, pW2)
    c0base = weights.pop("_c0base")

    zg = z[perm[:P2]]                       # [200000, 32] gathered pair rows
    in_maps = []
    for c in range(NCORES):
        im = {"x": _pack_core(zg[c * RPC : (c + 1) * RPC])}
        im.update(weights)
        in_maps.append(im)

    trace = bool(int(os.environ.get("KERNEL_TRACE", "0")))
    res = run_bass_kernel_spmd(nc, in_maps, list(range(NCORES)), trace=trace)
    LAST_RESULTS = res

    vb2f = np.asarray(vb2, np.float32)
    add_e = (vb2f - c0base)[None, :]
    add_o = (vb2f + c0base)[None, :]
    out = np.empty((B, 32), dtype=np.float32)
    og = np.empty((RPC * NCORES, 32), dtype=np.float32)
    for c in range(NCORES):
        ev, od = _unpack_core(res.results[c]["out"])
        blk = np.empty((NCHUNK, PAIRS_PC, 2, 32), dtype=np.float32)
        blk[:, :, 0] = ev[:, :PAIRS_PC] + add_e
        blk[:, :, 1] = od[:, :PAIRS_PC] + add_o
        og[c * RPC : (c + 1) * RPC] = blk.reshape(RPC, 32)
    out[perm[:P2]] = og
    out[perm[P2:]] = _host_triple(t, z[perm[P2:]], vW1, vb1, vW2, vb2, pW1, pb1, pW2)
    return out


# revision 13
# speedup vs baseline: 1.4382x; 1.0792x over previous
"""Trainium2 Bass kernel for nn_ODEFunc_interaction (gnn_message_passing).

Math (see reference):
  dz_dt = tanh([z, t] @ vW1 + vb1) @ vW2 + vb2                    (v-net, all rows)
  for each pair (perm[2i], perm[2i+1]):
      d_i  = z[perm[2i]] - z[perm[2i+1]]
      v    = tanh(d_i @ pW1 + pb1);  q = (pW1*pW2)^T v^2;  c0 = sum(pW1*pW2)
      out[perm[2i]]   = dz_dt[perm[2i]]   + q - c0   (+vb2)
      out[perm[2i+1]] = dz_dt[perm[2i+1]] - q + c0   (+vb2)
  last 3 rows (triple) handled on host in float64 (tiny).

Mapping: 8 cores, data-parallel over pairs. Per core 25000 rows = 12500 pairs,
packed 4 chunks deep in the partition dim: partition 32j+d = dim d of chunk j.
Column space (per chunk) = 3136 padded pairs in superblocks of 512 pairs
(tail 64), each superblock ordered [wave0-even 256 | wave0-odd 256 | w1-e |
w1-o].

Engine mapping:
 - PE, addressed as 32-wide tiles (tile_position): h/pa matmuls (K=32) as
   concurrent row-tiles (32j,0); dz/q (M=32) as concurrent col-tiles (0,32j)
   with shared stationaries. Each in-flight matmul owns a full PSUM bank
   (concurrent drains into one bank wedge the device). A warm-up burst of
   dummy matmuls during the initial DMA window brings the PE out of the
   HAM half-clock state before real work starts.
 - ACT computes only the h tanh (split in chunk-halves A/B so the psum
   ping-pong never stalls the ACT queue).
 - DVE computes tanh^2 for the pair branch in ONE custom op per chunk
   (deg-5 odd polynomial + clamp: min((y(c1+t(c2+t*c3)))^2, C), t=y^2,
   max err 0.022 -> q err ~0.014 abs, well within tolerance; pb1 is zero
   for this problem which frees the bias const slot). DVE also evacuates
   dz psum -> fp16 SBUF.
 - q accumulates into the dz psum region with +/- weights, so the final
   even/odd combine is free.
 - GPSIMD computes the pair diffs.
"""

import os
import numpy as np

B, D, H = 200003, 32, 128
NCORES = 8
P2 = 200000              # rows covered by pairs
RPC = P2 // NCORES       # 25000 rows per core
NCHUNK = 4
ROWS_PC = RPC // NCHUNK  # 6250 rows per chunk
PAIRS_PC = ROWS_PC // 2  # 3125 pairs per chunk
HALFP = 3136             # padded pairs per chunk: 6*512 + 64
SBW = 512                # pairs per superblock
SBS = [SBW] * 6 + [HALFP - 6 * SBW]
XC = 2 * HALFP           # 6272 columns per core

# deg-5-odd tanh^2 approx: min((y*(c1 + t*(c2 + t*c3)))^2, CLAMP), t = y*y
TSQ_C1, TSQ_C2, TSQ_C3 = 0.91987675, -0.17231731, 0.0153519
TSQ_CLAMP = 0.97771559

_CACHE = {}
LAST_RESULTS = None      # BassKernelResults of the most recent run (for test.py)


def _tanhsq_ref(in0, in1, s0, s1, imm2):
    x = in0.astype(np.float32)
    t = x * x
    v = x * (s0 + t * (s1 + t * imm2))
    return np.minimum(v * v, in1).astype(np.float32)


def _register_tanhsq():
    """Register the TANH_SQ_ANT custom-DVE op (8 uop stages)."""
    if "op" in _CACHE.setdefault("tanhsq", {}):
        return _CACHE["tanhsq"]["op"]
    from concourse import dve_ops
    from concourse.dve_spec import (
        Spec, Src0, C0, C1, C2, C3, minn, sq, _spill_c3_to_src1, lower, _has_src1,
    )
    from concourse.dve_uop import DveOpSpec

    name = "TANH_SQ_ANT"
    if name not in dve_ops._SUB_OPCODE_FOR_NAME:
        t = Src0 * Src0
        v = Src0 * (C0 + t * (C1 + t * C2))
        body = _spill_c3_to_src1(minn(sq(v), C3))
        spec = Spec(body=body, reference=_tanhsq_ref)
        row = 1 + len(dve_ops.OPS)
        assert row < 0x20
        dve_ops._SUB_OPCODE_FOR_NAME[name] = row
        shas = {}
        for ver in ("v3", "v4"):
            uops = lower(spec, ver=ver)
            shas[ver] = DveOpSpec(
                name=name, opcode=row, uops=uops, rd1_en=_has_src1(spec)
            ).sha(ver)
        op = dve_ops.DveOp(name, spec, subdim=False, uops_sha=shas)
        dve_ops.OPS.append(op)
        dve_ops.CUSTOM_DVE_SPECS[name] = spec
    else:
        op = next(o for o in dve_ops.OPS if o.name == name)
    _CACHE["tanhsq"]["op"] = op
    return op


def build_program(use_tanhsq=True):
    from contextlib import ExitStack
    import concourse.bacc as bacc
    import concourse.mybir as mybir
    import concourse.tile as tile

    tanhsq_op = _register_tanhsq() if use_tanhsq else None

    dt = mybir.dt
    F16, F32 = dt.float16, dt.float32
    AF = mybir.ActivationFunctionType
    OP = mybir.AluOpType

    nc = bacc.Bacc()
    X = nc.dram_tensor("x", [128, XC], F16, kind="ExternalInput")
    WT = nc.dram_tensor("wcat", [128, 352], F16, kind="ExternalInput")
    BT = nc.dram_tensor("bias", [128, 3], F32, kind="ExternalInput")
    O = nc.dram_tensor("out", [128, XC], F16, kind="ExternalOutput")

    with tile.TileContext(nc) as tc, ExitStack() as ctx:
        wpool = ctx.enter_context(tc.tile_pool(name="wpool", bufs=1))
        xpool = ctx.enter_context(tc.tile_pool(name="xpool", bufs=3))
        dfpool = ctx.enter_context(tc.tile_pool(name="dfpool", bufs=2))
        upool = ctx.enter_context(tc.tile_pool(name="upool", bufs=4))
        sqpool = ctx.enter_context(tc.tile_pool(name="sqpool", bufs=2))
        vpool = None
        if not use_tanhsq:
            vpool = ctx.enter_context(tc.tile_pool(name="vpool", bufs=2))
        opool = ctx.enter_context(tc.tile_pool(name="opool", bufs=3))
        # PSUM: ph 2x2 banks + pa 2x1 banks + dz 2x1 banks = 8 exactly
        hps = ctx.enter_context(tc.tile_pool(name="hps", bufs=2, space="PSUM"))
        papool = ctx.enter_context(tc.tile_pool(name="papool", bufs=2, space="PSUM"))
        dzpool = ctx.enter_context(tc.tile_pool(name="dzpool", bufs=2, space="PSUM"))

        # PE warm-up: ~4us of dummy matmuls on a zeroed tile, overlapping the
        # initial input DMA. Brings HAM out of the half-clock state so real
        # matmuls run at 2.4 GHz.
        zt = wpool.tile([128, 640], F16)
        nc.gpsimd.memset(zt[:], 0.0)
        warm = hps.tile([128, 1024], F32, tag="ph", name="warm")
        for _ in range(9):
            nc.tensor.matmul(zt_ps := warm[:, 0:512], zt[:, 0:128], zt[:, 128:640],
                             start=True, stop=True)

        # first input superblock before the (small) weight tensors: the x
        # transfer is the long pole for the first h matmuls
        xts = []
        xt0 = xpool.tile([128, 2 * SBW], F16, tag="xt", name="xt0")
        nc.sync.dma_start(xt0[:, :], X[:, 0 : 2 * SBW])

        wt = wpool.tile([128, 352], F16)
        nc.sync.dma_start(wt[:], WT[:])
        bt = wpool.tile([128, 3], F32)
        nc.sync.dma_start(bt[:], BT[:])
        w1 = wt[:, 0:128]      # [32j+d, h] = vW1[d, h]
        pw1 = wt[:, 128:256]   # [32j+d, h] = pW1[d, h]
        w2 = wt[:, 256:288]    # [h, d] = vW2[h, d]
        pwp = wt[:, 288:320]   # [h, d] = pW1[d, h] * pW2[h]
        pwn = wt[:, 320:352]   # -pwp
        bh = bt[:, 0:1]        # vb1 + t * vW1[32]
        pb1 = bt[:, 1:2]
        clamp = bt[:, 2:3]

        def h_half(xt, w, V, half, k):
            """h pre-acts for wave w, chunks (2*half, 2*half+1): 2 concurrent
            row-tiles, chunk c -> own psum bank; then one tanh -> fp16."""
            ph = hps.tile([128, 1024], F32, tag="ph", name=f"ph{k}_{w}_{half}")
            for c in range(2):
                j = 2 * half + c
                p0 = 32 * j
                nc.tensor.matmul(
                    ph[:, 512 * c : 512 * c + 2 * V],
                    w1[p0 : p0 + 32, :],
                    xt[p0 : p0 + 32, 512 * w : 512 * w + 2 * V],
                    start=True, stop=True,
                    tile_position=(p0, 0),
                )
            ut = upool.tile([128, 1024], F16, tag="ut", name=f"ut{k}_{w}_{half}")
            if V == 256:
                nc.scalar.activation(ut[:, :], ph[:, :], AF.Tanh, bias=bh[:])
            else:
                for c in range(2):
                    nc.scalar.activation(
                        ut[:, 512 * c : 512 * c + 2 * V],
                        ph[:, 512 * c : 512 * c + 2 * V],
                        AF.Tanh, bias=bh[:],
                    )
            return ut

        def pa_quarter(j, dft, sq, W, k):
            """pa pre-acts for chunk j -> [128,512] bank, then tanh^2 into
            the shared sq tile via the custom DVE op (or ACT fallback)."""
            paq = papool.tile([128, 512], F32, tag="paq", name=f"paq{k}_{j}")
            p0 = 32 * j
            nc.tensor.matmul(
                paq[:, 0:W],
                pw1[p0 : p0 + 32, :],
                dft[p0 : p0 + 32, :W],
                start=True, stop=True,
                tile_position=(p0, 0),
            )
            if use_tanhsq:
                nc.vector._custom_dve(
                    tanhsq_op,
                    out=sq[:, 512 * j : 512 * j + W],
                    in0=paq[:, 0:W],
                    in1=clamp,
                    s0=TSQ_C1, s1=TSQ_C2, imm2=TSQ_C3,
                )
            else:
                vt = vpool.tile([128, 512], F16, tag="vt", name=f"vt{k}_{j}")
                nc.scalar.activation(vt[:, 0:W], paq[:, 0:W], AF.Tanh, bias=pb1[:])
                nc.vector.tensor_tensor(
                    sq[:, 512 * j : 512 * j + W], vt[:, 0:W], vt[:, 0:W], OP.mult
                )

        c0 = 0
        for k, W in enumerate(SBS):
            W2_ = 2 * W
            waves = [(w, 256) for w in range(W // 256)] if W >= 256 else [(0, W)]
            if k == 0:
                xt = xt0
            else:
                xt = xpool.tile([128, 2 * SBW], F16, tag="xt", name=f"xt{k}")
                nc.sync.dma_start(xt[:, :W2_], X[:, c0 : c0 + W2_])

            # diffs, wave-major pair columns: df[:, 256w+i] = e - o
            dft = dfpool.tile([128, SBW], F16)
            for w, V in waves:
                nc.gpsimd.tensor_tensor(
                    dft[:, 256 * w : 256 * w + V],
                    xt[:, 512 * w : 512 * w + V],
                    xt[:, 512 * w + V : 512 * w + 2 * V],
                    OP.subtract,
                )

            # engine queue orders (independent per engine):
            #   PE : hA0 hB0 paQ01 hA1 hB1 paQ23 dz q
            #   ACT: thA0 thB0 thA1 thB1 (continuous; psum halves ping-pong)
            #   DVE: tanhsq 0..3, evac w0, evac w1
            sq = sqpool.tile([128, 2048], F16)
            uts = []
            w0, V0 = waves[0]
            utA0 = h_half(xt, w0, V0, 0, k)
            utB0 = h_half(xt, w0, V0, 1, k)
            uts.append((utA0, utB0))
            pa_quarter(0, dft, sq, W, k)
            pa_quarter(1, dft, sq, W, k)
            if len(waves) > 1:
                w1_, V1 = waves[1]
                utA1 = h_half(xt, w1_, V1, 0, k)
                utB1 = h_half(xt, w1_, V1, 1, k)
                uts.append((utA1, utB1))
            pa_quarter(2, dft, sq, W, k)
            pa_quarter(3, dft, sq, W, k)

            # dz + q accumulate per wave in [128,512] (bank per wave tile)
            for (w, V), (utA, utB) in zip(waves, uts):
                dzp = dzpool.tile([128, 512], F32, tag="dzp", name=f"dzp{k}_{w}")
                for j in range(NCHUNK):
                    p0 = 32 * j
                    utx = utA if j < 2 else utB
                    nc.tensor.matmul(
                        dzp[p0 : p0 + 32, 0 : 2 * V],
                        w2[:, :],
                        utx[:, 512 * (j % 2) : 512 * (j % 2) + 2 * V],
                        start=True, stop=False,
                        tile_position=(0, p0),
                        skip_group_check=True,
                    )
                for sgn, pw in ((0, pwp), (1, pwn)):
                    for j in range(NCHUNK):
                        p0 = 32 * j
                        nc.tensor.matmul(
                            dzp[p0 : p0 + 32, sgn * V : (sgn + 1) * V],
                            pw[:, :],
                            sq[:, 512 * j + 256 * w : 512 * j + 256 * w + V],
                            start=False, stop=(sgn == 1),
                            tile_position=(0, p0),
                            skip_group_check=True,
                        )
                ot = opool.tile([128, 512], F16, tag="ot", name=f"ot{k}_{w}")
                nc.vector.tensor_copy(ot[:, 0 : 2 * V], dzp[:, 0 : 2 * V])
                nc.sync.dma_start(O[:, c0 + 512 * w : c0 + 512 * w + 2 * V], ot[:, 0 : 2 * V])
            c0 += W2_

    nc.compile()
    return nc


def _prep_weights(t, vW1, vb1, vW2, vb2, pW1, pb1, pW2):
    f32 = np.float32
    t = np.asarray(t, dtype=f32).reshape(-1)[0]
    vW1 = np.asarray(vW1, f32)
    w1rep = np.tile(np.ascontiguousarray(vW1[:32]), (4, 1))            # [128,128]
    biash = (np.asarray(vb1, f32) + t * vW1[32]).reshape(128, 1).astype(f32)
    pW1 = np.asarray(pW1, f32)
    pw1rep = np.tile(pW1, (4, 1))                                      # [128,128]
    pb1c = np.asarray(pb1, f32).reshape(128, 1).copy()
    w2 = np.ascontiguousarray(np.asarray(vW2, f32))                    # [128,32]
    w2col = np.asarray(pW2, f32).reshape(128)
    pwp = np.ascontiguousarray(pW1.T * w2col[:, None])                 # [128,32]
    wcat = np.hstack([w1rep, pw1rep, w2, pwp, -pwp]).astype(np.float16)
    clampc = np.full((128, 1), TSQ_CLAMP, f32)
    bias = np.hstack([biash, pb1c, clampc]).astype(f32)
    # constant part of q: c0[d] = sum_h pW1[d,h]*pW2[h], in fp16 weight precision
    c0base = wcat[:, 288:320].astype(f32).sum(axis=0)                  # [32]
    return {"wcat": np.ascontiguousarray(wcat), "bias": np.ascontiguousarray(bias),
            "_c0base": c0base}


def _pack_core(zc):
    """[25000, 32] f32 -> [128, 6272] fp16: partition 32j+d = dim d of chunk j;
    cols per superblock: [w0-even 256 | w0-odd 256 | w1-e 256 | w1-o 256]."""
    zp = np.zeros((NCHUNK, HALFP, 2, 32), dtype=np.float16)
    zp[:, :PAIRS_PC] = zc.reshape(NCHUNK, PAIRS_PC, 2, 32)
    # full superblocks: [4, 6, 2(wave), 256, 2(half), 32]
    full = zp[:, : 6 * SBW].reshape(NCHUNK, 6, 2, 256, 2, 32)
    out = np.empty((128, XC), dtype=np.float16)
    out[:, : 12 * SBW] = full.transpose(0, 5, 1, 2, 4, 3).reshape(128, 12 * SBW)
    tail = zp[:, 6 * SBW :]                                 # [4, 64, 2, 32]
    out[:, 12 * SBW :] = tail.transpose(0, 3, 2, 1).reshape(128, 2 * (HALFP - 6 * SBW))
    return out


def _unpack_core(oc):
    """[128, 6272] fp16 -> even [4,HALFP,32], odd [4,HALFP,32] (f32)."""
    T = HALFP - 6 * SBW
    full = (
        oc[:, : 12 * SBW].astype(np.float32)
        .reshape(NCHUNK, 32, 6, 2, 2, 256)
        .transpose(0, 2, 3, 5, 4, 1)                        # [4, 6, wave, 256, half, 32]
        .reshape(NCHUNK, 6 * SBW, 2, 32)
    )
    tail = (
        oc[:, 12 * SBW :].astype(np.float32)
        .reshape(NCHUNK, 32, 2, T)
        .transpose(0, 3, 2, 1)                              # [4, T, 2, 32]
    )
    ev = np.concatenate([full[:, :, 0], tail[:, :, 0]], axis=1)
    od = np.concatenate([full[:, :, 1], tail[:, :, 1]], axis=1)
    return ev, od


def _host_triple(t, z3, vW1, vb1, vW2, vb2, pW1, pb1, pW2):
    """Exact float64 computation of the 3 leftover rows: dz_dt + triple forces."""
    f8 = np.float64
    z3 = z3.astype(f8)
    vW1 = np.asarray(vW1, f8)
    t = float(np.asarray(t).reshape(-1)[0])
    h3 = np.tanh(z3 @ vW1[:32] + t * vW1[32] + np.asarray(vb1, f8))
    dz3 = h3 @ np.asarray(vW2, f8) + np.asarray(vb2, f8)

    pW1 = np.asarray(pW1, f8)
    w2 = np.asarray(pW2, f8).reshape(128)
    d9 = (z3[:, None, :] - z3[None, :, :]).reshape(9, 32)
    u9 = np.tanh(d9 @ pW1 + np.asarray(pb1, f8))
    s9 = (1.0 - u9 * u9) * w2[None, :]
    g9 = s9 @ pW1.T
    f9 = (-g9).reshape(3, 3, 32)
    f9 = f9 * (1.0 - np.eye(3)[:, :, None])
    force3 = f9.sum(axis=1) * 2.0
    return (dz3 + force3).astype(np.float32)


def kernel(t, z, perm, vW1, vb1, vW2, vb2, pW1, pb1, pW2, pb2):
    from concourse.bass_utils import run_bass_kernel_spmd

    global LAST_RESULTS
    use_tanhsq = bool(np.all(np.asarray(pb1) == 0))
    key = ("nc", use_tanhsq)
    if key not in _CACHE:
        _CACHE[key] = build_program(use_tanhsq)
    nc = _CACHE[key]

    z = np.asarray(z, np.float32)
    perm = np.asarray(perm)
    weights = _prep_weights(t, vW1, vb1, vW2, vb2, pW1, pb1, pW2)
    c0base = weights.pop("_c0base")

    zg = z[perm[:P2]]                       # [200000, 32] gathered pair rows
    in_maps = []
    for c in range(NCORES):
        im = {"x": _pack_core(zg[c * RPC : (c + 1) * RPC])}
        im.update(weights)
        in_maps.append(im)

    trace = bool(int(os.environ.get("KERNEL_TRACE", "0")))
    res = run_bass_kernel_spmd(nc, in_maps, list(range(NCORES)), trace=trace)
    LAST_RESULTS = res

    vb2f = np.asarray(vb2, np.float32)
    add_e = (vb2f - c0base)[None, :]
    add_o = (vb2f + c0base)[None, :]
    out = np.empty((B, 32), dtype=np.float32)
    og = np.empty((RPC * NCORES, 32), dtype=np.float32)
    for c in range(NCORES):
        ev, od = _unpack_core(res.results[c]["out"])
        blk = np.empty((NCHUNK, PAIRS_PC, 2, 32), dtype=np.float32)
        blk[:, :, 0] = ev[:, :PAIRS_PC] + add_e
        blk[:, :, 1] = od[:, :PAIRS_PC] + add_o
        og[c * RPC : (c + 1) * RPC] = blk.reshape(RPC, 32)
    out[perm[:P2]] = og
    out[perm[P2:]] = _host_triple(t, z[perm[P2:]], vW1, vb1, vW2, vb2, pW1, pb1, pW2)
    return out
